# revision 4
# baseline (speedup 1.0000x reference)
"""Fused attention block (LGHIFusion) for Trainium2, 8-core tensor-parallel.

Math (per reference):
  Q = low  @ W_Q.T + b_Q ; K = low @ W_K.T + b_K ; V = high @ W_V.T + b_V
  attn = softmax(Q K^T / sqrt(dh)) ; ctx = attn @ V
  Z = ctx @ W_O.T + b_O ; out = low + sigmoid(gamma) * Z

Sharding: tensor-parallel over heads. 16 heads / 8 cores = 2 heads/core.
Each core computes QT/KT/VT for its 128 output dims, per-head attention
with scores kept TRANSPOSED ([k, q] layout) so softmax denominators come
free from an appended ones-column in V (no PE transposes of P needed),
then its partial Z = ctx @ W_O[:, shard].T (full 1024 output dims).
Host sums the 8 fp16 partials and applies residual + beta*b_O.

Perf structure:
 - Inputs DMAed in 512KB per-k-block-per-batch transfers (near peak HBM
   bw); weights pre-laid-out on host so each is one contiguous DMA.
 - Phase D k-tile loop is software-pipelined: scores(kt+1) is emitted
   BEFORE ctx(kt) so the in-order PE queue never stalls on the ACT exp
   of tile kt; phase D runs at the ACT (exp) roofline.
 - All matmuls bf16 (full PE rate, FWL); fp16 partials out. The
   beta=sigmoid(-5)~0.0067 gate damps kernel error ~150x in the final
   output, so bf16/fp16 error is small end to end.
"""

import numpy as np

try:
    import concourse.bass as bass
except ImportError:  # pragma: no cover
    import sys

    sys.path.insert(0, "/opt/trn_rl_repo")
    import concourse.bass as bass

import concourse.mybir as mybir
from concourse.bass_utils import run_bass_kernel_spmd
from concourse.masks import make_identity
from concourse.tile import TileContext

dt = mybir.dt
F32, BF16, F16 = dt.float32, dt.bfloat16, dt.float16
AF = mybir.ActivationFunctionType

B, S, D = 2, 2048, 1024
H, DH = 16, 64
T = B * S            # 4096 tokens
NCORES = 8
HPC = H // NCORES    # 2 heads per core
OPC = HPC * DH       # 128 out dims per core
VW = DH + 1          # V columns + ones column = 65
KT_N = S // 128      # 16 k-tiles per batch
NKT = T // 128       # 32 global token tiles
PCH = 512            # projection token-chunk size
QC = 1024            # q-chunk for attention
ND = D // 128        # 8 contraction blocks


def _build_nc(rep=1):
    # rep>1 wraps the whole body in a hardware loop (bench-only: amplifies
    # exec time over the dispatch floor for timing; graded path uses rep=1).
    nc = bass.Bass("TRN2", target_bir_lowering=False, debug=False,
                   num_devices=NCORES)

    xt_lo = nc.dram_tensor("xt_lo", [D, T], BF16, kind="ExternalInput").ap()
    xt_hi = nc.dram_tensor("xt_hi", [D, T], BF16, kind="ExternalInput").ap()
    # Weights pre-arranged on host to the exact SBUF image [128, D].
    wq_t = nc.dram_tensor("wq_t", [128, D], BF16, kind="ExternalInput").ap()
    wk_t = nc.dram_tensor("wk_t", [128, D], BF16, kind="ExternalInput").ap()
    wv_t = nc.dram_tensor("wv_t", [128, D], BF16, kind="ExternalInput").ap()
    wo_t = nc.dram_tensor("wo_t", [OPC, D], BF16, kind="ExternalInput").ap()
    bq_d = nc.dram_tensor("bq", [1, OPC], BF16, kind="ExternalInput").ap()
    bk_d = nc.dram_tensor("bk", [1, OPC], BF16, kind="ExternalInput").ap()
    bv_d = nc.dram_tensor("bv", [1, OPC], BF16, kind="ExternalInput").ap()
    z_out = nc.dram_tensor("z_out", [T, D], F16, kind="ExternalOutput").ap()

    with TileContext(nc) as tc:
        with (
            tc.tile_pool(name="const", bufs=1) as const,
            tc.tile_pool(name="w", bufs=1) as wpool,
            tc.tile_pool(name="x", bufs=2) as xpool,
            tc.tile_pool(name="acts", bufs=1) as actpool,
            tc.tile_pool(name="vone", bufs=1) as vpool,
            tc.tile_pool(name="pt", bufs=3) as ptpool,
            tc.tile_pool(name="ctxn", bufs=2) as cxpool,
            tc.tile_pool(name="z16", bufs=3) as zpool,
            tc.tile_pool(name="r", bufs=2) as rpool,
            tc.tile_pool(name="ps", bufs=2, space="PSUM") as pp,
            tc.tile_pool(name="pc", bufs=1, space="PSUM") as pc,
        ):
          import contextlib
          loop_cm = tc.For_i(0, rep, 1) if rep > 1 else contextlib.nullcontext()
          with loop_cm:
            # ---- Phase A: weights (single contiguous DMAs), constants ----
            wq = wpool.tile([128, D], BF16, tag="wq")
            wk = wpool.tile([128, D], BF16, tag="wk")
            wv = wpool.tile([128, D], BF16, tag="wv")
            wo = wpool.tile([128, D], BF16, tag="wo")
            nc.sync.dma_start(wq[:], wq_t[:, :])
            nc.sync.dma_start(wk[:], wk_t[:, :])
            nc.sync.dma_start(wv[:], wv_t[:, :])
            nc.sync.dma_start(wo[:], wo_t[:, :])
            bq = const.tile([1, OPC], BF16, tag="bq")
            bk = const.tile([1, OPC], BF16, tag="bk")
            bv = const.tile([1, OPC], BF16, tag="bv")
            nc.sync.dma_start(bq[:], bq_d[:, :])
            nc.sync.dma_start(bk[:], bk_d[:, :])
            nc.sync.dma_start(bv[:], bv_d[:, :])

            ident = const.tile([128, 128], BF16)
            make_identity(nc, ident[:])
            ones_p = const.tile([1, PCH], BF16, tag="ones_p")
            nc.vector.memset(ones_p[:], 1.0)
            ones64 = const.tile([1, DH], F32, tag="ones64")
            nc.vector.memset(ones64[:], 1.0)

            # Persistent activations: [128 outdims, token] transposed layout.
            qt = actpool.tile([128, T], BF16, tag="qt")
            kts = actpool.tile([128, T], BF16, tag="kt")
            vts = actpool.tile([128, T], BF16, tag="vt")
            # V in [k, dh] layout + ones column per (ktile, head).
            vone = vpool.tile([128, NKT * HPC * VW], BF16)
            nc.vector.memset(vone[:], 1.0)

            # ---- Phase B: per-batch x loads (512KB DMAs) + projections ----
            for b in range(B):
                xlo = xpool.tile([128, ND * S], BF16, tag="xlo")
                xhi = xpool.tile([128, ND * S], BF16, tag="xhi")
                for k in range(ND):
                    nc.sync.dma_start(
                        xlo[:, S * k:S * (k + 1)],
                        xt_lo[128 * k:128 * (k + 1), b * S:(b + 1) * S])
                for k in range(ND):
                    nc.sync.dma_start(
                        xhi[:, S * k:S * (k + 1)],
                        xt_hi[128 * k:128 * (k + 1), b * S:(b + 1) * S])
                # Q,K first (only need xlo), then V (needs xhi, which
                # streams in while Q/K compute).
                for wmat, bias, dest, src in (
                    (wq, bq, qt, xlo),
                    (wk, bk, kts, xlo),
                    (wv, bv, vts, xhi),
                ):
                    for tch in range(S // PCH):
                        t0 = tch * PCH
                        ps = pp.tile([128, PCH], F32, tag="s")
                        for k in range(ND):
                            nc.tensor.matmul(
                                ps[:],
                                lhsT=wmat[:, 128 * k:128 * (k + 1)],
                                rhs=src[:, S * k + t0:S * k + t0 + PCH],
                                start=(k == 0), stop=False)
                        nc.tensor.matmul(ps[:], lhsT=bias[:], rhs=ones_p[:],
                                         start=False, stop=True)
                        nc.vector.tensor_copy(
                            dest[:, b * S + t0:b * S + t0 + PCH], ps[:])

            # ---- Phase C: V -> [k, dh] via PE transpose, into vone ----
            for g in range(NKT):
                pt_ps = pc.tile([128, 128], BF16, tag="c")
                nc.tensor.transpose(pt_ps[:], vts[:, 128 * g:128 * (g + 1)],
                                    ident[:])
                for h in range(HPC):
                    base = (g * HPC + h) * VW
                    nc.vector.tensor_copy(vone[:, base:base + DH],
                                          pt_ps[:, DH * h:DH * (h + 1)])

            # ---- Phase D: attention, scores transposed [k, q] ----
            # Software-pipelined: scores(kt+1) is emitted before ctx(kt) so
            # PE runs ahead of the ACT exp instead of stalling each k-tile.
            for b in range(B):
                ctxn = cxpool.tile([128, S], BF16)
                for h in range(HPC):
                    hp = DH * h
                    for qc in range(S // QC):
                        q0 = b * S + qc * QC

                        def emit_scores(kt):
                            g = b * KT_N + kt
                            ps_s = pp.tile([128, QC], F32, tag="s")
                            for hf in range(QC // 512):
                                nc.tensor.matmul(
                                    ps_s[:, 512 * hf:512 * (hf + 1)],
                                    lhsT=kts[hp:hp + DH,
                                             128 * g:128 * (g + 1)],
                                    rhs=qt[hp:hp + DH,
                                           q0 + 512 * hf:q0 + 512 * (hf + 1)],
                                    start=True, stop=True)
                            return ps_s

                        ps_c = pc.tile([VW, QC], F32, tag="c")
                        ps_prev = emit_scores(0)
                        for kt in range(KT_N):
                            ps_next = (emit_scores(kt + 1)
                                       if kt + 1 < KT_N else None)
                            pt = ptpool.tile([128, QC], BF16)
                            nc.scalar.activation(pt[:], ps_prev[:], AF.Exp,
                                                 scale=0.125)
                            vbase = ((b * KT_N + kt) * HPC + h) * VW
                            for hf in range(QC // 512):
                                nc.tensor.matmul(
                                    ps_c[:, 512 * hf:512 * (hf + 1)],
                                    lhsT=vone[:, vbase:vbase + VW],
                                    rhs=pt[:, 512 * hf:512 * (hf + 1)],
                                    start=(kt == 0), stop=(kt == KT_N - 1))
                            ps_prev = ps_next

                        recip = rpool.tile([1, QC], F32, tag="recip")
                        nc.vector.reciprocal(recip[:], ps_c[DH:DH + 1, :])
                        ps_bc = pc.tile([DH, QC], F32, tag="bc")
                        for hf in range(QC // 512):
                            nc.tensor.matmul(
                                ps_bc[:, 512 * hf:512 * (hf + 1)],
                                lhsT=ones64[:],
                                rhs=recip[:, 512 * hf:512 * (hf + 1)],
                                start=True, stop=True)
                        bc_sb = rpool.tile([DH, QC], F32, tag="bc")
                        nc.vector.tensor_copy(bc_sb[:], ps_bc[:])
                        nc.vector.tensor_mul(
                            ctxn[hp:hp + DH, qc * QC:(qc + 1) * QC],
                            ps_c[0:DH, :], bc_sb[:])

                # ---- Phase E: partial Z = ctxN.T @ W_O_shard.T ----
                for qt_i in range(S // 128):
                    ps_z = pp.tile([128, D], F32, tag="s")
                    for hf in range(D // 512):
                        nc.tensor.matmul(
                            ps_z[:, 512 * hf:512 * (hf + 1)],
                            lhsT=ctxn[:, 128 * qt_i:128 * (qt_i + 1)],
                            rhs=wo[:, 512 * hf:512 * (hf + 1)],
                            start=True, stop=True)
                    z16 = zpool.tile([128, D], F16)
                    nc.vector.tensor_copy(z16[:], ps_z[:])
                    r0 = b * S + 128 * qt_i
                    nc.sync.dma_start(z_out[r0:r0 + 128, :], z16[:])

    _split_waits(nc)
    return nc


def _split_waits(nc):
    """This walrus build accepts only one sync-wait per instruction.
    Move extra waits onto same-engine NoOps inserted just before each
    offender (engine program order preserves the gating)."""
    for f in nc.m.functions:
        for blk in f.blocks:
            new_insts = []
            for inst in blk.instructions:
                si = inst.sync_info
                if si is not None and si.on_wait and len(si.on_wait) > 1:
                    waits = list(si.on_wait)
                    for w in waits[:-1]:
                        nop = mybir.InstNoOp(
                            name=nc.get_next_instruction_name(),
                            sync_info=mybir.SyncInfo(on_wait=[w],
                                                     on_update=[]),
                            bass_nofuse=True,
                            engine=inst.engine,
                        )
                        new_insts.append(nop)
                    si.on_wait = [waits[-1]]
                new_insts.append(inst)
            blk.instructions[:] = new_insts


_NC_CACHE = None


def _get_nc():
    global _NC_CACHE
    if _NC_CACHE is None:
        _NC_CACHE = _build_nc()
    return _NC_CACHE


def _sb_weight(Wl):
    """[128, 1024] weight -> the SBUF lhsT image: out[p, 128k+o] =
    Wl[o, 128k+p] (contraction block k on partitions, out dim on cols)."""
    return np.ascontiguousarray(
        Wl.reshape(128, ND, 128).transpose(2, 1, 0).reshape(128, D))


def _make_in_maps(inputs):
    low = np.ascontiguousarray(np.asarray(inputs["low_freq"], np.float32))
    high = np.ascontiguousarray(np.asarray(inputs["high_freq"], np.float32))
    W_Q = np.asarray(inputs["W_Q"], np.float32)
    W_K = np.asarray(inputs["W_K"], np.float32)
    W_V = np.asarray(inputs["W_V"], np.float32)
    W_O = np.asarray(inputs["W_O"], np.float32)
    b_Q = np.asarray(inputs["b_Q"], np.float32)
    b_K = np.asarray(inputs["b_K"], np.float32)
    b_V = np.asarray(inputs["b_V"], np.float32)

    import ml_dtypes
    bf16 = ml_dtypes.bfloat16
    xt_lo = np.ascontiguousarray(low.reshape(T, D).T.astype(bf16))
    xt_hi = np.ascontiguousarray(high.reshape(T, D).T.astype(bf16))

    in_maps = []
    for c in range(NCORES):
        sl = slice(OPC * c, OPC * (c + 1))
        in_maps.append({
            "xt_lo": xt_lo,
            "xt_hi": xt_hi,
            "wq_t": _sb_weight(W_Q[sl, :]).astype(bf16),
            "wk_t": _sb_weight(W_K[sl, :]).astype(bf16),
            "wv_t": _sb_weight(W_V[sl, :]).astype(bf16),
            "wo_t": np.ascontiguousarray(W_O[:, sl].T.astype(bf16)),
            "bq": np.ascontiguousarray(b_Q[sl].reshape(1, OPC).astype(bf16)),
            "bk": np.ascontiguousarray(b_K[sl].reshape(1, OPC).astype(bf16)),
            "bv": np.ascontiguousarray(b_V[sl].reshape(1, OPC).astype(bf16)),
        })
    return in_maps


def _run(inputs, trace=False, **kw):
    low = np.ascontiguousarray(np.asarray(inputs["low_freq"], np.float32))
    b_O = np.asarray(inputs["b_O"], np.float32)
    gamma = float(np.asarray(inputs["gamma"], np.float32))
    in_maps = _make_in_maps(inputs)

    nc = _get_nc()
    res = run_bass_kernel_spmd(nc, in_maps, list(range(NCORES)), trace=trace,
                               **kw)

    zsum = np.zeros((T, D), np.float32)
    for r in res.results:
        zsum += r["z_out"].astype(np.float32)
    beta = 1.0 / (1.0 + np.exp(-gamma))
    out = low.reshape(T, D) + beta * (zsum + b_O[None, :])
    return out.reshape(B, S, D), res


def kernel(**inputs):
    out, _ = _run(inputs)
    return out


# revision 13
# speedup vs baseline: 11.5985x; 11.5985x over previous
"""Fused attention block (LGHIFusion) for Trainium2, 8-core tensor-parallel.

Math (per reference):
  Q = low  @ W_Q.T + b_Q ; K = low @ W_K.T + b_K ; V = high @ W_V.T + b_V
  attn = softmax(Q K^T / sqrt(dh)) ; ctx = attn @ V
  Z = ctx @ W_O.T + b_O ; out = low + sigmoid(gamma) * Z

Sharding: tensor-parallel over heads. 16 heads / 8 cores = 2 heads/core.
Each core computes QT/KT/VT for its 128 output dims, per-head attention
with scores kept TRANSPOSED ([k, q] layout) so softmax denominators come
free from an appended ones-column in V (no PE transposes of P needed),
then its partial Z = ctx @ W_O[:, shard].T (full 1024 output dims).
Host sums the 8 fp16 partials and applies residual + beta*b_O.

Perf structure:
 - Inputs DMAed in 512KB per-k-block-per-batch transfers (near peak HBM
   bw); weights pre-laid-out on host so each is one contiguous DMA.
 - Phase D k-tile loop is software-pipelined: scores(kt+1) is emitted
   BEFORE ctx(kt) so the in-order PE queue never stalls on the ACT exp
   of tile kt; phase D runs at the ACT (exp) roofline.
 - All matmuls bf16 (full PE rate, FWL); fp16 partials out. The
   beta=sigmoid(-5)~0.0067 gate damps kernel error ~150x in the final
   output, so bf16/fp16 error is small end to end.
"""

import numpy as np

try:
    import concourse.bass as bass
except ImportError:  # pragma: no cover
    import sys

    sys.path.insert(0, "/opt/trn_rl_repo")
    import concourse.bass as bass

import concourse.mybir as mybir
from concourse.bass_utils import run_bass_kernel_spmd
from concourse.masks import make_identity
from concourse.tile import TileContext

dt = mybir.dt
F32, BF16, F16 = dt.float32, dt.bfloat16, dt.float16
AF = mybir.ActivationFunctionType

B, S, D = 2, 2048, 1024
H, DH = 16, 64
T = B * S            # 4096 tokens
NCORES = 8
HPC = H // NCORES    # 2 heads per core
OPC = HPC * DH       # 128 out dims per core
VW = DH + 1          # V columns + ones column = 65
KT_N = S // 128      # 16 k-tiles per batch
NKT = T // 128       # 32 global token tiles
PCH = 512            # projection token-chunk size
QC = 512             # q-chunk for attention
ND = D // 128        # 8 contraction blocks


def _build_nc(rep=1):
    # rep>1 wraps the whole body in a hardware loop (bench-only: amplifies
    # exec time over the dispatch floor for timing; graded path uses rep=1).
    nc = bass.Bass("TRN2", target_bir_lowering=False, debug=False,
                   num_devices=NCORES)

    xt_lo = nc.dram_tensor("xt_lo", [D, T], BF16, kind="ExternalInput").ap()
    xt_hi = nc.dram_tensor("xt_hi", [D, T], BF16, kind="ExternalInput").ap()
    # Weights pre-arranged on host to the exact SBUF image [128, D].
    wq_t = nc.dram_tensor("wq_t", [128, D], BF16, kind="ExternalInput").ap()
    wk_t = nc.dram_tensor("wk_t", [128, D], BF16, kind="ExternalInput").ap()
    wv_t = nc.dram_tensor("wv_t", [128, D], BF16, kind="ExternalInput").ap()
    wo_t = nc.dram_tensor("wo_t", [OPC, D], BF16, kind="ExternalInput").ap()
    bq_d = nc.dram_tensor("bq", [1, OPC], BF16, kind="ExternalInput").ap()
    bk_d = nc.dram_tensor("bk", [1, OPC], BF16, kind="ExternalInput").ap()
    bv_d = nc.dram_tensor("bv", [1, OPC], BF16, kind="ExternalInput").ap()
    z_out = nc.dram_tensor("z_out", [T, D], F16, kind="ExternalOutput").ap()

    with TileContext(nc) as tc:
        with (
            tc.tile_pool(name="const", bufs=1) as const,
            tc.tile_pool(name="w", bufs=1) as wpool,
            tc.tile_pool(name="x", bufs=2) as xpool,
            tc.tile_pool(name="acts", bufs=1) as actpool,
            tc.tile_pool(name="vone", bufs=1) as vpool,
            tc.tile_pool(name="pt", bufs=3) as ptpool,
            tc.tile_pool(name="ctxn", bufs=2) as cxpool,
            tc.tile_pool(name="z16", bufs=3) as zpool,
            tc.tile_pool(name="r", bufs=2) as rpool,
            tc.tile_pool(name="ps", bufs=2, space="PSUM") as pp,
            tc.tile_pool(name="pc", bufs=2, space="PSUM") as pc,
        ):
          import contextlib
          loop_cm = tc.For_i(0, rep, 1) if rep > 1 else contextlib.nullcontext()
          with loop_cm:
            # ---- Phase A: weights (single contiguous DMAs), constants ----
            wq = wpool.tile([128, D], BF16, tag="wq")
            wk = wpool.tile([128, D], BF16, tag="wk")
            wv = wpool.tile([128, D], BF16, tag="wv")
            wo = wpool.tile([128, D], BF16, tag="wo")
            nc.sync.dma_start(wq[:], wq_t[:, :])
            nc.sync.dma_start(wk[:], wk_t[:, :])
            nc.sync.dma_start(wv[:], wv_t[:, :])
            nc.sync.dma_start(wo[:], wo_t[:, :])
            bq = const.tile([1, OPC], BF16, tag="bq")
            bk = const.tile([1, OPC], BF16, tag="bk")
            bv = const.tile([1, OPC], BF16, tag="bv")
            nc.sync.dma_start(bq[:], bq_d[:, :])
            nc.sync.dma_start(bk[:], bk_d[:, :])
            nc.sync.dma_start(bv[:], bv_d[:, :])

            ident = const.tile([128, 128], BF16)
            make_identity(nc, ident[:])
            ones_p = const.tile([1, PCH], BF16, tag="ones_p")
            nc.vector.memset(ones_p[:], 1.0)
            ones64 = const.tile([1, DH], F32, tag="ones64")
            nc.vector.memset(ones64[:], 1.0)

            # Persistent activations: [128 outdims, token] transposed layout.
            qt = actpool.tile([128, T], BF16, tag="qt")
            kts = actpool.tile([128, T], BF16, tag="kt")
            vts = actpool.tile([128, T], BF16, tag="vt")
            # V in [k, dh] layout + ones column per (ktile, head).
            vone = vpool.tile([128, NKT * HPC * VW], BF16)
            nc.vector.memset(vone[:], 1.0)

            # ---- x loads: per-batch 512KB DMAs, issued up front ----
            xbufs = []
            for b in range(B):
                xlo = xpool.tile([128, ND * S], BF16, tag="xlo")
                xhi = xpool.tile([128, ND * S], BF16, tag="xhi")
                for k in range(ND):
                    nc.sync.dma_start(
                        xlo[:, S * k:S * (k + 1)],
                        xt_lo[128 * k:128 * (k + 1), b * S:(b + 1) * S])
                for k in range(ND):
                    nc.sync.dma_start(
                        xhi[:, S * k:S * (k + 1)],
                        xt_hi[128 * k:128 * (k + 1), b * S:(b + 1) * S])
                xbufs.append((xlo, xhi))

            # ---- Emitters (phases B/C/E as small PE work-parcels that can
            # be slotted into phase D's ACT-bound k-tile loop) ----
            def proj_emitters(b, mats="qkv", chunks=None):
                """Projections for batch b: each group split in two ~1us
                parcels (4-5 matmuls) so the s-ring is never held across
                more than 2 slots."""
                xlo, xhi = xbufs[b]
                sel = {"q": (wq, bq, qt, xlo), "k": (wk, bk, kts, xlo),
                       "v": (wv, bv, vts, xhi)}
                for wmat, bias, dest, src in (sel[m] for m in mats):
                    for tch in (range(S // PCH) if chunks is None
                                else chunks):
                        t0 = tch * PCH
                        st = {}

                        def part1(wmat=wmat, src=src, t0=t0, st=st):
                            ps = pp.tile([128, PCH], F32, tag="s")
                            st["ps"] = ps
                            for k in range(4):
                                nc.tensor.matmul(
                                    ps[:],
                                    lhsT=wmat[:, 128 * k:128 * (k + 1)],
                                    rhs=src[:, S * k + t0:S * k + t0 + PCH],
                                    start=(k == 0), stop=False)

                        def part2(wmat=wmat, bias=bias, dest=dest, src=src,
                                  t0=t0, st=st, b=b):
                            ps = st["ps"]
                            for k in range(4, ND):
                                nc.tensor.matmul(
                                    ps[:],
                                    lhsT=wmat[:, 128 * k:128 * (k + 1)],
                                    rhs=src[:, S * k + t0:S * k + t0 + PCH],
                                    start=False, stop=False)
                            nc.tensor.matmul(ps[:], lhsT=bias[:],
                                             rhs=ones_p[:],
                                             start=False, stop=True)
                            nc.vector.tensor_copy(
                                dest[:, b * S + t0:b * S + t0 + PCH], ps[:])

                        yield part1
                        yield part2

            def transp_emitters(b):
                """Phase C for batch b: V -> [k, dh] via PE transpose."""
                for kt in range(KT_N):
                    def emit(kt=kt, b=b):
                        g = b * KT_N + kt
                        pt_ps = pp.tile([128, 128], BF16, tag="s")
                        nc.tensor.transpose(
                            pt_ps[:], vts[:, 128 * g:128 * (g + 1)],
                            ident[:])
                        for h in range(HPC):
                            base = (g * HPC + h) * VW
                            nc.vector.tensor_copy(
                                vone[:, base:base + DH],
                                pt_ps[:, DH * h:DH * (h + 1)])
                    yield emit

            def z_emitters(b, ctxn, lo, hi):
                """Phase E z-tiles [lo, hi) for batch b."""
                for qt_i in range(lo, hi):
                    def emit(qt_i=qt_i, b=b, ctxn=ctxn):
                        ps_z = pp.tile([128, D], F32, tag="s")
                        for hf in range(D // 512):
                            nc.tensor.matmul(
                                ps_z[:, 512 * hf:512 * (hf + 1)],
                                lhsT=ctxn[:, 128 * qt_i:128 * (qt_i + 1)],
                                rhs=wo[:, 512 * hf:512 * (hf + 1)],
                                start=True, stop=True)
                        z16 = zpool.tile([128, D], F16)
                        nc.vector.tensor_copy(z16[:], ps_z[:])
                        r0 = b * S + 128 * qt_i
                        nc.sync.dma_start(z_out[r0:r0 + 128, :], z16[:])
                    yield emit

            # ---- Serial prologue: K,V projections + V-transposes for
            # batch 0, plus only the FIRST Q chunk (Q for unit u is only
            # needed when unit u starts; later chunks become gated fillers
            # inside D(b0)). ----
            for em in proj_emitters(0, mats="k"):
                em()
            for em in proj_emitters(0, mats="q", chunks=[0]):
                em()
            for em in proj_emitters(0, mats="v"):
                em()
            for em in transp_emitters(0):
                em()

            # ---- Phase D: attention, scores transposed [k, q] ----
            # Per (b, qc) unit both heads are processed together:
            #  - the two heads' score matmuls (contraction 64) sit in PE
            #    array rows 0-63 / 64-127 (tile_position auto-derived) and
            #    run CONCURRENTLY;
            #  - scores land in a [128, 2*QC] bf16 PSUM pair-tile (2 banks)
            #    so ONE exp covers both heads (N=2048 amortizes ACT
            #    overhead);
            #  - software-pipelined: scores(kt+1) emitted before ctx(kt) so
            #    the in-order PE queue never stalls on ACT.
            NU = S // QC          # qc-units per batch
            ZPU = (S // 128) // NU  # z-tiles per qc-unit
            ctxns = [None, None]
            leftover = []

            def run_d_batch(b, fillers, defer_z=False, unit_pre=None):
                """Phase D for batch b; pops one filler parcel after each
                k-tile's emissions (PE slack under the ACT-bound exp)."""
                ctxn = cxpool.tile([128, S], BF16)
                ctxns[b] = ctxn
                fq = list(fillers)
                fi = [0]

                def pop_filler():
                    if fi[0] < len(fq):
                        fq[fi[0]]()
                        fi[0] += 1

                for qc in range(NU):
                    if unit_pre and qc in unit_pre:
                        fq[fi[0]:fi[0]] = unit_pre[qc]
                    q0 = b * S + qc * QC

                    def emit_scores(kt):
                        g = b * KT_N + kt
                        ps_s = pp.tile([128, 2 * QC], F32, tag="s")
                        for h in range(HPC):
                            hp = DH * h
                            nc.tensor.matmul(
                                ps_s[:, QC * h:QC * (h + 1)],
                                lhsT=kts[hp:hp + DH, 128 * g:128 * (g + 1)],
                                rhs=qt[hp:hp + DH, q0:q0 + QC],
                                start=True, stop=True)
                        return ps_s

                    ps_c0 = pc.tile([VW, QC], F32, tag="c0")
                    ps_c1 = pc.tile([VW, QC], F32, tag="c1")
                    ps_c = [ps_c0, ps_c1]
                    ps_prev = emit_scores(0)
                    for kt in range(KT_N):
                        ps_next = (emit_scores(kt + 1)
                                   if kt + 1 < KT_N else None)
                        pt = ptpool.tile([128, 2 * QC], BF16)
                        nc.scalar.activation(pt[:], ps_prev[:], AF.Exp,
                                             scale=0.125)
                        for h in range(HPC):
                            vbase = ((b * KT_N + kt) * HPC + h) * VW
                            for hf in range(QC // 512):
                                nc.tensor.matmul(
                                    ps_c[h][:, 512 * hf:512 * (hf + 1)],
                                    lhsT=vone[:, vbase:vbase + VW],
                                    rhs=pt[:, QC * h + 512 * hf:
                                           QC * h + 512 * (hf + 1)],
                                    start=(kt == 0), stop=(kt == KT_N - 1))
                        ps_prev = ps_next
                        pop_filler()

                    for h in range(HPC):
                        hp = DH * h
                        recip = rpool.tile([1, QC], F32, tag="recip")
                        nc.vector.reciprocal(recip[:], ps_c[h][DH:DH + 1, :])
                        ps_bc = pp.tile([DH, QC], F32, tag="s")
                        for hf in range(QC // 512):
                            nc.tensor.matmul(
                                ps_bc[:, 512 * hf:512 * (hf + 1)],
                                lhsT=ones64[:],
                                rhs=recip[:, 512 * hf:512 * (hf + 1)],
                                start=True, stop=True)
                        bc_sb = rpool.tile([DH, QC], F32, tag="bc")
                        nc.vector.tensor_copy(bc_sb[:], ps_bc[:])
                        nc.vector.tensor_mul(
                            ctxn[hp:hp + DH, qc * QC:(qc + 1) * QC],
                            ps_c[h][0:DH, :], bc_sb[:])

                    # Phase E parcels for the just-finished unit: batch 0's
                    # all go to D(b1)'s filler queue (D(b0) already hosts
                    # B+C of batch 1); batch 1's feed the NEXT unit here.
                    if defer_z or qc + 1 == NU:
                        leftover.extend(z_emitters(b, ctxn, ZPU * qc,
                                                   ZPU * (qc + 1)))
                    else:
                        fq.extend(z_emitters(b, ctxn, ZPU * qc,
                                             ZPU * (qc + 1)))
                # drain unconsumed fillers
                while fi[0] < len(fq):
                    pop_filler()

            # D(b0) hosts B+C for batch 1; D(b1) hosts the leftover E(b0)
            # tiles; E(b1)'s last unit drains after D(b1).
            q_pre = {u: list(proj_emitters(0, mats="q", chunks=[u + 1]))
                     for u in range(NU - 1)}
            run_d_batch(0, list(proj_emitters(1)) + list(transp_emitters(1)),
                        defer_z=True, unit_pre=q_pre)
            lo0 = list(leftover)
            leftover.clear()
            run_d_batch(1, lo0)
            for em in leftover:
                em()

    _split_waits(nc)
    return nc


def _split_waits(nc):
    """This walrus build accepts only one sync-wait per instruction.
    Move extra waits onto same-engine NoOps inserted just before each
    offender (engine program order preserves the gating)."""
    for f in nc.m.functions:
        for blk in f.blocks:
            new_insts = []
            for inst in blk.instructions:
                si = inst.sync_info
                if si is not None and si.on_wait and len(si.on_wait) > 1:
                    waits = list(si.on_wait)
                    for w in waits[:-1]:
                        nop = mybir.InstNoOp(
                            name=nc.get_next_instruction_name(),
                            sync_info=mybir.SyncInfo(on_wait=[w],
                                                     on_update=[]),
                            bass_nofuse=True,
                            engine=inst.engine,
                        )
                        new_insts.append(nop)
                    si.on_wait = [waits[-1]]
                new_insts.append(inst)
            blk.instructions[:] = new_insts


_NC_CACHE = None


def _get_nc():
    global _NC_CACHE
    if _NC_CACHE is None:
        _NC_CACHE = _build_nc()
    return _NC_CACHE


def _sb_weight(Wl):
    """[128, 1024] weight -> the SBUF lhsT image: out[p, 128k+o] =
    Wl[o, 128k+p] (contraction block k on partitions, out dim on cols)."""
    return np.ascontiguousarray(
        Wl.reshape(128, ND, 128).transpose(2, 1, 0).reshape(128, D))


def _make_in_maps(inputs):
    low = np.ascontiguousarray(np.asarray(inputs["low_freq"], np.float32))
    high = np.ascontiguousarray(np.asarray(inputs["high_freq"], np.float32))
    W_Q = np.asarray(inputs["W_Q"], np.float32)
    W_K = np.asarray(inputs["W_K"], np.float32)
    W_V = np.asarray(inputs["W_V"], np.float32)
    W_O = np.asarray(inputs["W_O"], np.float32)
    b_Q = np.asarray(inputs["b_Q"], np.float32)
    b_K = np.asarray(inputs["b_K"], np.float32)
    b_V = np.asarray(inputs["b_V"], np.float32)

    import ml_dtypes
    bf16 = ml_dtypes.bfloat16
    xt_lo = np.ascontiguousarray(low.reshape(T, D).T.astype(bf16))
    xt_hi = np.ascontiguousarray(high.reshape(T, D).T.astype(bf16))

    in_maps = []
    for c in range(NCORES):
        sl = slice(OPC * c, OPC * (c + 1))
        in_maps.append({
            "xt_lo": xt_lo,
            "xt_hi": xt_hi,
            "wq_t": _sb_weight(W_Q[sl, :]).astype(bf16),
            "wk_t": _sb_weight(W_K[sl, :]).astype(bf16),
            "wv_t": _sb_weight(W_V[sl, :]).astype(bf16),
            "wo_t": np.ascontiguousarray(W_O[:, sl].T.astype(bf16)),
            "bq": np.ascontiguousarray(b_Q[sl].reshape(1, OPC).astype(bf16)),
            "bk": np.ascontiguousarray(b_K[sl].reshape(1, OPC).astype(bf16)),
            "bv": np.ascontiguousarray(b_V[sl].reshape(1, OPC).astype(bf16)),
        })
    return in_maps


def _run(inputs, trace=False, **kw):
    low = np.ascontiguousarray(np.asarray(inputs["low_freq"], np.float32))
    b_O = np.asarray(inputs["b_O"], np.float32)
    gamma = float(np.asarray(inputs["gamma"], np.float32))
    in_maps = _make_in_maps(inputs)

    nc = _get_nc()
    res = run_bass_kernel_spmd(nc, in_maps, list(range(NCORES)), trace=trace,
                               **kw)

    zsum = np.zeros((T, D), np.float32)
    for r in res.results:
        zsum += r["z_out"].astype(np.float32)
    beta = 1.0 / (1.0 + np.exp(-gamma))
    out = low.reshape(T, D) + beta * (zsum + b_O[None, :])
    return out.reshape(B, S, D), res


def kernel(**inputs):
    out, _ = _run(inputs)
    return out


# revision 15
# speedup vs baseline: 12.8093x; 1.1044x over previous
"""Fused attention block (LGHIFusion) for Trainium2, 8-core tensor-parallel.

Math (per reference):
  Q = low  @ W_Q.T + b_Q ; K = low @ W_K.T + b_K ; V = high @ W_V.T + b_V
  attn = softmax(Q K^T / sqrt(dh)) ; ctx = attn @ V
  Z = ctx @ W_O.T + b_O ; out = low + sigmoid(gamma) * Z

Sharding: tensor-parallel over heads. 16 heads / 8 cores = 2 heads/core.
Each core computes QT/KT/VT for its 128 output dims, per-head attention
with scores kept TRANSPOSED ([k, q] layout) so softmax denominators come
free from an appended ones-column in V (no PE transposes of P needed),
then its partial Z = ctx @ W_O[:, shard].T (full 1024 output dims).
Host sums the 8 fp16 partials and applies residual + beta*b_O.

Perf structure:
 - Inputs DMAed in 512KB per-k-block-per-batch transfers (near peak HBM
   bw); weights pre-laid-out on host so each is one contiguous DMA.
 - Phase D k-tile loop is software-pipelined: scores(kt+1) is emitted
   BEFORE ctx(kt) so the in-order PE queue never stalls on the ACT exp
   of tile kt; phase D runs at the ACT (exp) roofline.
 - All matmuls bf16 (full PE rate, FWL); fp16 partials out. The
   beta=sigmoid(-5)~0.0067 gate damps kernel error ~150x in the final
   output, so bf16/fp16 error is small end to end.
"""

import numpy as np

try:
    import concourse.bass as bass
except ImportError:  # pragma: no cover
    import sys

    sys.path.insert(0, "/opt/trn_rl_repo")
    import concourse.bass as bass

import concourse.mybir as mybir
from concourse.bass_utils import run_bass_kernel_spmd
from concourse.masks import make_identity
from concourse.tile import TileContext

dt = mybir.dt
F32, BF16, F16 = dt.float32, dt.bfloat16, dt.float16
AF = mybir.ActivationFunctionType

B, S, D = 2, 2048, 1024
H, DH = 16, 64
T = B * S            # 4096 tokens
NCORES = 8
HPC = H // NCORES    # 2 heads per core
OPC = HPC * DH       # 128 out dims per core
VW = DH + 1          # V columns + ones column = 65
KT_N = S // 128      # 16 k-tiles per batch
NKT = T // 128       # 32 global token tiles
PCH = 512            # projection token-chunk size
QC = 512             # q-chunk for attention
ND = D // 128        # 8 contraction blocks


def _build_nc(rep=1):
    # rep>1 wraps the whole body in a hardware loop (bench-only: amplifies
    # exec time over the dispatch floor for timing; graded path uses rep=1).
    nc = bass.Bass("TRN2", target_bir_lowering=False, debug=False,
                   num_devices=NCORES)

    xt_lo = nc.dram_tensor("xt_lo", [D, T], BF16, kind="ExternalInput").ap()
    xt_hi = nc.dram_tensor("xt_hi", [D, T], BF16, kind="ExternalInput").ap()
    # Weights pre-arranged on host to the exact SBUF image [128, D].
    wq_t = nc.dram_tensor("wq_t", [128, D], BF16, kind="ExternalInput").ap()
    wk_t = nc.dram_tensor("wk_t", [128, D], BF16, kind="ExternalInput").ap()
    wv_t = nc.dram_tensor("wv_t", [128, D], BF16, kind="ExternalInput").ap()
    wo_t = nc.dram_tensor("wo_t", [OPC, D], BF16, kind="ExternalInput").ap()
    bq_d = nc.dram_tensor("bq", [1, OPC], BF16, kind="ExternalInput").ap()
    bk_d = nc.dram_tensor("bk", [1, OPC], BF16, kind="ExternalInput").ap()
    bv_d = nc.dram_tensor("bv", [1, OPC], BF16, kind="ExternalInput").ap()
    z_out = nc.dram_tensor("z_out", [T, D], F16, kind="ExternalOutput").ap()

    with TileContext(nc) as tc:
        with (
            tc.tile_pool(name="const", bufs=1) as const,
            tc.tile_pool(name="w", bufs=1) as wpool,
            tc.tile_pool(name="x", bufs=2) as xpool,
            tc.tile_pool(name="acts", bufs=1) as actpool,
            tc.tile_pool(name="vone", bufs=1) as vpool,
            tc.tile_pool(name="pt", bufs=3) as ptpool,
            tc.tile_pool(name="ctxn", bufs=2) as cxpool,
            tc.tile_pool(name="z16", bufs=3) as zpool,
            tc.tile_pool(name="r", bufs=2) as rpool,
            tc.tile_pool(name="ps", bufs=2, space="PSUM") as pp,
            tc.tile_pool(name="pc", bufs=1, space="PSUM") as pc,
        ):
          import contextlib
          loop_cm = tc.For_i(0, rep, 1) if rep > 1 else contextlib.nullcontext()
          with loop_cm:
            # ---- Phase A: weights (single contiguous DMAs), constants ----
            wq = wpool.tile([128, D], BF16, tag="wq")
            wk = wpool.tile([128, D], BF16, tag="wk")
            wv = wpool.tile([128, D], BF16, tag="wv")
            wo = wpool.tile([128, D], BF16, tag="wo")
            nc.sync.dma_start(wq[:], wq_t[:, :])
            nc.sync.dma_start(wk[:], wk_t[:, :])
            nc.sync.dma_start(wv[:], wv_t[:, :])
            nc.sync.dma_start(wo[:], wo_t[:, :])
            bq = const.tile([1, OPC], BF16, tag="bq")
            bk = const.tile([1, OPC], BF16, tag="bk")
            bv = const.tile([1, OPC], BF16, tag="bv")
            nc.sync.dma_start(bq[:], bq_d[:, :])
            nc.sync.dma_start(bk[:], bk_d[:, :])
            nc.sync.dma_start(bv[:], bv_d[:, :])

            ident = const.tile([128, 128], BF16)
            make_identity(nc, ident[:])
            ones_p = const.tile([1, PCH], BF16, tag="ones_p")
            nc.vector.memset(ones_p[:], 1.0)
            ones64 = const.tile([1, DH], F32, tag="ones64")
            nc.vector.memset(ones64[:], 1.0)

            # Persistent activations: [128 outdims, token] transposed layout.
            qt = actpool.tile([128, T], BF16, tag="qt")
            kts = actpool.tile([128, T], BF16, tag="kt")
            vts = actpool.tile([128, T], BF16, tag="vt")
            # V in [k, dh] layout + ones column per (ktile, head).
            vone = vpool.tile([128, NKT * HPC * VW], BF16)
            nc.vector.memset(vone[:], 1.0)

            # ---- x loads: per-batch 512KB DMAs, issued up front ----
            xbufs = []
            for b in range(B):
                xlo = xpool.tile([128, ND * S], BF16, tag="xlo")
                xhi = xpool.tile([128, ND * S], BF16, tag="xhi")
                if b == 0:
                    # Token-major chunked loads: projection chunk tch only
                    # needs columns [512tch, 512tch+512) of every k-block,
                    # so the first chunk is DMA-complete after ~1MB.
                    for src_d, dst in ((xt_lo, xlo), (xt_hi, xhi)):
                        for tch in range(S // PCH):
                            t0 = tch * PCH
                            for k in range(ND):
                                nc.sync.dma_start(
                                    dst[:, S * k + t0:S * k + t0 + PCH],
                                    src_d[128 * k:128 * (k + 1),
                                          t0:t0 + PCH])
                else:
                    for k in range(ND):
                        nc.sync.dma_start(
                            xlo[:, S * k:S * (k + 1)],
                            xt_lo[128 * k:128 * (k + 1), b * S:(b + 1) * S])
                    for k in range(ND):
                        nc.sync.dma_start(
                            xhi[:, S * k:S * (k + 1)],
                            xt_hi[128 * k:128 * (k + 1), b * S:(b + 1) * S])
                xbufs.append((xlo, xhi))

            # ---- Emitters (phases B/C/E as small PE work-parcels that can
            # be slotted into phase D's ACT-bound k-tile loop) ----
            def proj_emitters(b, mats="qkv", chunks=None):
                """Projections for batch b: each group split in two ~1us
                parcels (4-5 matmuls) so the s-ring is never held across
                more than 2 slots."""
                xlo, xhi = xbufs[b]
                sel = {"q": (wq, bq, qt, xlo), "k": (wk, bk, kts, xlo),
                       "v": (wv, bv, vts, xhi)}
                for wmat, bias, dest, src in (sel[m] for m in mats):
                    for tch in (range(S // PCH) if chunks is None
                                else chunks):
                        t0 = tch * PCH
                        st = {}

                        def part1(wmat=wmat, src=src, t0=t0, st=st):
                            ps = pp.tile([128, PCH], F32, tag="f")
                            st["ps"] = ps
                            for k in range(4):
                                nc.tensor.matmul(
                                    ps[:],
                                    lhsT=wmat[:, 128 * k:128 * (k + 1)],
                                    rhs=src[:, S * k + t0:S * k + t0 + PCH],
                                    start=(k == 0), stop=False)

                        def part2(wmat=wmat, bias=bias, dest=dest, src=src,
                                  t0=t0, st=st, b=b):
                            ps = st["ps"]
                            for k in range(4, ND):
                                nc.tensor.matmul(
                                    ps[:],
                                    lhsT=wmat[:, 128 * k:128 * (k + 1)],
                                    rhs=src[:, S * k + t0:S * k + t0 + PCH],
                                    start=False, stop=False)
                            nc.tensor.matmul(ps[:], lhsT=bias[:],
                                             rhs=ones_p[:],
                                             start=False, stop=True)
                            nc.vector.tensor_copy(
                                dest[:, b * S + t0:b * S + t0 + PCH], ps[:])

                        yield part1
                        yield part2

            def transp_emitters(b):
                """Phase C for batch b: V -> [k, dh] via PE transpose."""
                for kt in range(KT_N):
                    def emit(kt=kt, b=b):
                        g = b * KT_N + kt
                        pt_ps = pp.tile([128, 128], BF16, tag="f")
                        nc.tensor.transpose(
                            pt_ps[:], vts[:, 128 * g:128 * (g + 1)],
                            ident[:])
                        for h in range(HPC):
                            base = (g * HPC + h) * VW
                            nc.vector.tensor_copy(
                                vone[:, base:base + DH],
                                pt_ps[:, DH * h:DH * (h + 1)])
                    yield emit

            def z_emitters(b, ctxn, lo, hi):
                """Phase E z-tiles [lo, hi) for batch b, as two ~650ns
                half-tile parcels each (1-bank PSUM slices of tag "f")."""
                for qt_i in range(lo, hi):
                    st = {}

                    def zp1(qt_i=qt_i, ctxn=ctxn, st=st):
                        ps_z = pp.tile([128, 512], F32, tag="f")
                        nc.tensor.matmul(
                            ps_z[:],
                            lhsT=ctxn[:, 128 * qt_i:128 * (qt_i + 1)],
                            rhs=wo[:, 0:512], start=True, stop=True)
                        z16 = zpool.tile([128, D], F16)
                        st["z"] = z16
                        nc.vector.tensor_copy(z16[:, 0:512], ps_z[:])

                    def zp2(qt_i=qt_i, b=b, ctxn=ctxn, st=st):
                        ps_z = pp.tile([128, 512], F32, tag="f")
                        nc.tensor.matmul(
                            ps_z[:],
                            lhsT=ctxn[:, 128 * qt_i:128 * (qt_i + 1)],
                            rhs=wo[:, 512:1024], start=True, stop=True)
                        z16 = st["z"]
                        nc.vector.tensor_copy(z16[:, 512:1024], ps_z[:])
                        r0 = b * S + 128 * qt_i
                        nc.sync.dma_start(z_out[r0:r0 + 128, :], z16[:])

                    yield zp1
                    yield zp2

            # ---- Serial prologue: K,V projections + V-transposes for
            # batch 0, plus only the FIRST Q chunk (Q for unit u is only
            # needed when unit u starts; later chunks become gated fillers
            # inside D(b0)). ----
            for em in proj_emitters(0, mats="k"):
                em()
            for em in proj_emitters(0, mats="q", chunks=[0]):
                em()
            for em in proj_emitters(0, mats="v"):
                em()
            for em in transp_emitters(0):
                em()

            # ---- Phase D: one continuous software-pipelined stream over
            # all (batch, q-chunk) units x k-tiles.
            #  - the two heads' score matmuls (contraction 64) sit in PE
            #    array rows 0-63 / 64-127 (tile_position auto-derived) and
            #    run CONCURRENTLY; scores land in a [128, 2*QC] f32 pair-
            #    tile so ONE exp covers both heads;
            #  - scores(slot i+1) emitted before ctx(slot i): the in-order
            #    PE queue never stalls on ACT, and the pipeline does NOT
            #    break at unit/batch boundaries (tails are emitted one slot
            #    into the next unit);
            #  - the "s" PSUM ring carries ONLY score pair-tiles (pure
            #    depth-2 pipeline); fillers (proj/transpose/bc/z parcels)
            #    rotate through their own "f" ring.
            NU = S // QC            # qc-units per batch
            ZPU = (S // 128) // NU  # z-tiles per qc-unit
            units = [(u // NU, u % NU) for u in range(B * NU)]
            ctxn0 = cxpool.tile([128, S], BF16, tag="cx")
            ctxn1 = cxpool.tile([128, S], BF16, tag="cx")
            ctxns = [ctxn0, ctxn1]

            fq = list(proj_emitters(1)) + list(transp_emitters(1))
            fi = [0]

            def pop_filler():
                if fi[0] < len(fq):
                    fq[fi[0]]()
                    fi[0] += 1

            def emit_scores(u, kt):
                b, qc = units[u]
                g = b * KT_N + kt
                q0 = b * S + qc * QC
                ps_s = pp.tile([128, 2 * QC], F32, tag="s")
                for h in range(HPC):
                    hp = DH * h
                    nc.tensor.matmul(
                        ps_s[:, QC * h:QC * (h + 1)],
                        lhsT=kts[hp:hp + DH, 128 * g:128 * (g + 1)],
                        rhs=qt[hp:hp + DH, q0:q0 + QC],
                        start=True, stop=True)
                return ps_s

            ps_cs = {}

            def emit_tail(u):
                b, qc = units[u]
                ctxn = ctxns[b]
                for h in range(HPC):
                    hp = DH * h
                    recip = rpool.tile([1, QC], F32, tag="recip")
                    nc.vector.reciprocal(recip[:], ps_cs[u][h][DH:DH + 1, :])
                    ps_bc = pp.tile([DH, QC], F32, tag="f")
                    nc.tensor.matmul(ps_bc[:], lhsT=ones64[:], rhs=recip[:],
                                     start=True, stop=True)
                    bc_sb = rpool.tile([DH, QC], F32, tag="bc")
                    nc.vector.tensor_copy(bc_sb[:], ps_bc[:])
                    nc.vector.tensor_mul(
                        ctxn[hp:hp + DH, qc * QC:(qc + 1) * QC],
                        ps_cs[u][h][0:DH, :], bc_sb[:])
                del ps_cs[u]
                # this unit's z-output parcels are now data-ready
                fq.extend(z_emitters(b, ctxn, ZPU * qc, ZPU * (qc + 1)))

            slots = [(u, kt) for u in range(len(units)) for kt in range(KT_N)]
            ps_prev = emit_scores(0, 0)
            for i, (u, kt) in enumerate(slots):
                b, qc = units[u]
                if kt == 0:
                    # batch-0's Q chunk qc+1 is first needed by unit qc+1:
                    # feed it through the filler queue head during unit qc.
                    if b == 0 and qc + 1 < NU:
                        fq[fi[0]:fi[0]] = list(
                            proj_emitters(0, mats="q", chunks=[qc + 1]))
                    ps_c0 = pc.tile([VW, QC], F32, tag="c0")
                    ps_c1 = pc.tile([VW, QC], F32, tag="c1")
                    ps_cs[u] = [ps_c0, ps_c1]
                ps_next = (emit_scores(*slots[i + 1])
                           if i + 1 < len(slots) else None)
                pt = ptpool.tile([128, 2 * QC], BF16)
                nc.scalar.activation(pt[:], ps_prev[:], AF.Exp, scale=0.125)
                if kt == 0 and u > 0:
                    emit_tail(u - 1)
                for h in range(HPC):
                    vbase = ((b * KT_N + kt) * HPC + h) * VW
                    nc.tensor.matmul(
                        ps_cs[u][h][:],
                        lhsT=vone[:, vbase:vbase + VW],
                        rhs=pt[:, QC * h:QC * (h + 1)],
                        start=(kt == 0), stop=(kt == KT_N - 1))
                ps_prev = ps_next
                pop_filler()
            emit_tail(len(units) - 1)
            while fi[0] < len(fq):
                pop_filler()

    _split_waits(nc)
    return nc


def _split_waits(nc):
    """This walrus build accepts only one sync-wait per instruction.
    Move extra waits onto same-engine NoOps inserted just before each
    offender (engine program order preserves the gating)."""
    for f in nc.m.functions:
        for blk in f.blocks:
            new_insts = []
            for inst in blk.instructions:
                si = inst.sync_info
                if si is not None and si.on_wait and len(si.on_wait) > 1:
                    waits = list(si.on_wait)
                    for w in waits[:-1]:
                        nop = mybir.InstNoOp(
                            name=nc.get_next_instruction_name(),
                            sync_info=mybir.SyncInfo(on_wait=[w],
                                                     on_update=[]),
                            bass_nofuse=True,
                            engine=inst.engine,
                        )
                        new_insts.append(nop)
                    si.on_wait = [waits[-1]]
                new_insts.append(inst)
            blk.instructions[:] = new_insts


_NC_CACHE = None


def _get_nc():
    global _NC_CACHE
    if _NC_CACHE is None:
        _NC_CACHE = _build_nc()
    return _NC_CACHE


def _sb_weight(Wl):
    """[128, 1024] weight -> the SBUF lhsT image: out[p, 128k+o] =
    Wl[o, 128k+p] (contraction block k on partitions, out dim on cols)."""
    return np.ascontiguousarray(
        Wl.reshape(128, ND, 128).transpose(2, 1, 0).reshape(128, D))


def _make_in_maps(inputs):
    low = np.ascontiguousarray(np.asarray(inputs["low_freq"], np.float32))
    high = np.ascontiguousarray(np.asarray(inputs["high_freq"], np.float32))
    W_Q = np.asarray(inputs["W_Q"], np.float32)
    W_K = np.asarray(inputs["W_K"], np.float32)
    W_V = np.asarray(inputs["W_V"], np.float32)
    W_O = np.asarray(inputs["W_O"], np.float32)
    b_Q = np.asarray(inputs["b_Q"], np.float32)
    b_K = np.asarray(inputs["b_K"], np.float32)
    b_V = np.asarray(inputs["b_V"], np.float32)

    import ml_dtypes
    bf16 = ml_dtypes.bfloat16
    xt_lo = np.ascontiguousarray(low.reshape(T, D).T.astype(bf16))
    xt_hi = np.ascontiguousarray(high.reshape(T, D).T.astype(bf16))

    in_maps = []
    for c in range(NCORES):
        sl = slice(OPC * c, OPC * (c + 1))
        in_maps.append({
            "xt_lo": xt_lo,
            "xt_hi": xt_hi,
            "wq_t": _sb_weight(W_Q[sl, :]).astype(bf16),
            "wk_t": _sb_weight(W_K[sl, :]).astype(bf16),
            "wv_t": _sb_weight(W_V[sl, :]).astype(bf16),
            "wo_t": np.ascontiguousarray(W_O[:, sl].T.astype(bf16)),
            "bq": np.ascontiguousarray(b_Q[sl].reshape(1, OPC).astype(bf16)),
            "bk": np.ascontiguousarray(b_K[sl].reshape(1, OPC).astype(bf16)),
            "bv": np.ascontiguousarray(b_V[sl].reshape(1, OPC).astype(bf16)),
        })
    return in_maps


def _run(inputs, trace=False, **kw):
    low = np.ascontiguousarray(np.asarray(inputs["low_freq"], np.float32))
    b_O = np.asarray(inputs["b_O"], np.float32)
    gamma = float(np.asarray(inputs["gamma"], np.float32))
    in_maps = _make_in_maps(inputs)

    nc = _get_nc()
    res = run_bass_kernel_spmd(nc, in_maps, list(range(NCORES)), trace=trace,
                               **kw)

    zsum = np.zeros((T, D), np.float32)
    for r in res.results:
        zsum += r["z_out"].astype(np.float32)
    beta = 1.0 / (1.0 + np.exp(-gamma))
    out = low.reshape(T, D) + beta * (zsum + b_O[None, :])
    return out.reshape(B, S, D), res


def kernel(**inputs):
    out, _ = _run(inputs)
    return out
